# revision 22
# baseline (speedup 1.0000x reference)
"""DeepSpeedAttention (B=2, S=2048, H=4096, 32 heads) on 8 Trainium2 cores.

Sharding: tensor-parallel across heads. Each core computes QKV for its 4
heads (column shard of attn_qkvw), full attention for those heads, and a
partial output projection (row shard of attn_ow). The 8 bf16 partial outputs
are summed on the host (host-side all-reduce) in f32 and the bias is added.

Schedule (per core) - four segments, engineered so TensorE never waits on
the ACT-bound softmax:
  seg0: QKV projection for batch 0 (token chunks 0-3). Classic weight-
        stationary sweep; warmup matmuls + chunked first DMAs hide HAM
        cold-start; (b0,h0) attention head loads are emitted here.
  seg1: QKV projection for batch 1 FUSED with the whole of batch-0
        attention: the attention work is emitted as small "pieces"
        (2 k-tiles: scores matmuls + 512-wide exp + lagged PV) between
        projection matmul groups, so exp/DVE latency hides under dense
        independent projection work. x is streamed in half-token (N=256)
        double-buffered tiles to make room for resident wq/wk/wv.
  seg2: batch-1 attention (classic 1024-wide exp, PV lagged one k-pair
        globally) with batch-0 out-projection tiles interleaved as PE
        filler.
  seg3: remaining out-projection tail on a deep PSUM ring (matmul-bound).
Softmax denominator: probs pair-sums fold progressively into t4/t2 on DVE
(bf16), then one ones-matrix matmul group does the 128-partition
reduce+broadcast; reciprocal+scale pipelined into the next chunk.
"""

import os
import numpy as np
import ml_dtypes
from contextlib import ExitStack

try:
    import jax
    jax.config.update(
        "jax_compilation_cache_dir", os.path.expanduser("~/.bass_jax_cache"))
    jax.config.update("jax_persistent_cache_min_compile_time_secs", 10.0)
    jax.config.update("jax_persistent_cache_min_entry_size_bytes", 0)
except Exception:
    pass

import concourse.bass as bass
from concourse import bass_isa
import concourse.tile as tile
from concourse import bacc, mybir
from concourse.bass_utils import run_bass_kernel_spmd

BF16 = mybir.dt.bfloat16
F32 = mybir.dt.float32
AF = mybir.ActivationFunctionType

H = 4096          # hidden
TOK = 4096        # B*S tokens
S = 2048          # seq len per batch
NB = 2            # batches
HL = 4            # heads per core
HD = 128          # head dim
COLS = HL * HD    # per-core hidden shard (512)
NCORES = 8
KT = H // 128     # 32 contraction tiles for the projections
NKT = S // 128    # 16 k tiles per batch
SCALE = 1.0 / float(np.sqrt(HD))


def build_nc(phases: str = "ABC"):
    nc = bacc.Bacc("TRN2", target_bir_lowering=False, debug=False)

    xT = nc.dram_tensor("xT", [H, TOK], BF16, kind="ExternalInput").ap()
    wq = nc.dram_tensor("wq", [H, COLS], BF16, kind="ExternalInput").ap()
    wk = nc.dram_tensor("wk", [H, COLS], BF16, kind="ExternalInput").ap()
    wv = nc.dram_tensor("wv", [H, COLS], BF16, kind="ExternalInput").ap()
    bq = nc.dram_tensor("bq", [1, COLS], F32, kind="ExternalInput").ap()
    bk = nc.dram_tensor("bk", [1, COLS], F32, kind="ExternalInput").ap()
    bv = nc.dram_tensor("bv", [1, COLS], BF16, kind="ExternalInput").ap()
    wo = nc.dram_tensor("wo", [COLS, H], BF16, kind="ExternalInput").ap()
    # bf16 partials (summed in f32 on host): halves out-DMA + eviction cost
    out = nc.dram_tensor("out", [TOK, H], BF16, kind="ExternalOutput").ap()

    with tile.TileContext(nc) as tc, ExitStack() as ctx:
        # staging split per batch so batch-0 head loads can start mid-phase-A
        dram = ctx.enter_context(tc.tile_pool(name="dram", bufs=1, space="DRAM"))
        qT_d = [dram.tile([COLS, S], BF16, name=f"qT_d{b}") for b in range(NB)]
        kT_d = [dram.tile([COLS, S], BF16, name=f"kT_d{b}") for b in range(NB)]
        v_d = [dram.tile([S, COLS], BF16, name=f"v_d{b}") for b in range(NB)]

        const = ctx.enter_context(tc.tile_pool(name="const", bufs=1))
        ones_bf = const.tile([1, 512], BF16)
        nc.vector.memset(ones_bf[:], 1.0)
        # [128,128] all-ones: one matmul does the 128-partition denominator
        # reduce AND broadcasts the sum to every output partition
        onesmat = const.tile([128, 128], BF16)
        nc.vector.memset(onesmat[:], 1.0)
        warm_sb = const.tile([128, 128], BF16)
        nc.vector.memset(warm_sb[:], 0.125)
        # per-partition layout [col-within-tile, col-tile] for tensor_scalar
        bq_sb = const.tile([128, HL], F32)
        nc.sync.dma_start(bq_sb[:], bq.rearrange("o (ct p) -> p (o ct)", p=128))
        bk_sb = const.tile([128, HL], F32)
        nc.sync.dma_start(bk_sb[:], bk.rearrange("o (ct p) -> p (o ct)", p=128))
        bv_sb = const.tile([1, COLS], BF16)
        nc.sync.dma_start(bv_sb[:], bv)

        # head-tile pool that survives segment boundaries: (b0,h0) loads are
        # emitted mid-seg0, (b1,h0) loads mid-seg1
        bqk0 = ctx.enter_context(tc.tile_pool(name="bqk0", bufs=1))
        b0h0 = {}
        b1h0 = {}

        def load_head(pool, b, hl, store, suffix):
            r0 = hl * 128
            qh = pool.tile([128, S], BF16, tag="qh", name=f"qh{suffix}")
            nc.sync.dma_start(qh[:], qT_d[b][r0:r0 + 128, :])
            kh = pool.tile([128, S], BF16, tag="kh", name=f"kh{suffix}")
            nc.sync.dma_start(kh[:], kT_d[b][r0:r0 + 128, :])
            vh = pool.tile([128, NKT, 128], BF16, tag="vh", name=f"vh{suffix}")
            nc.sync.dma_start(
                vh[:],
                v_d[b][:, r0:r0 + 128].rearrange("(i p) d -> p i d", p=128))
            store.clear()
            store.update({"qh": qh, "kh": kh, "vh": vh})

        # ---------------- seg0: QKV for batch 0 ----------------
        if "A" in phases:
         with tc.tile_pool(name="aw", bufs=1) as awp, \
             tc.tile_pool(name="ax", bufs=2) as axp, \
             tc.tile_pool(name="ast", bufs=6) as astp, \
             tc.tile_pool(name="aps", bufs=4, space="PSUM") as apsp:
            # PE warmup: dead back-to-back matmuls during the initial DMA
            # wait get HAM to K=8/8 before the first real matmul issues
            wps = apsp.tile([128, 512], F32, tag="qk")
            for _ in range(64):
                nc.tensor.matmul(
                    wps[:, 0:128], warm_sb[:], warm_sb[:], start=True, stop=True)
            # wq + first x chunk first, in kt-chunks so the first q matmuls
            # can start after ~1/4 of the bytes land (subtile deps)
            wq_sb = awp.tile([128, KT, COLS], BF16)
            x0_sb = axp.tile([128, KT, 512], BF16, tag="x")
            for c in range(4):
                r0, r1, k0, k1 = c * 1024, (c + 1) * 1024, c * 8, (c + 1) * 8
                nc.sync.dma_start(
                    wq_sb[:, k0:k1, :],
                    wq[r0:r1, :].rearrange("(kt p) c -> p kt c", p=128))
                nc.sync.dma_start(
                    x0_sb[:, k0:k1, :],
                    xT[r0:r1, 0:512].rearrange("(kt p) t -> p kt t", p=128))
            wk_sb = awp.tile([128, KT, COLS], BF16)
            nc.sync.dma_start(wk_sb[:], wk.rearrange("(kt p) c -> p kt c", p=128))
            wv_sb = awp.tile([128, KT, COLS], BF16)
            nc.sync.dma_start(wv_sb[:], wv.rearrange("(kt p) c -> p kt c", p=128))

            for tck in range(4):
                t0 = tck * 512
                if tck == 0:
                    x_sb = x0_sb
                else:
                    x_sb = axp.tile([128, KT, 512], BF16, tag="x")
                    nc.sync.dma_start(
                        x_sb[:],
                        xT[:, t0:t0 + 512].rearrange("(kt p) t -> p kt t", p=128),
                    )
                # qT / kT: [col-tile 128, tok 512], weight stationary
                if tck == 0:
                    # kt-chunk-major for the q groups: 4 accumulation groups
                    # open at once so compute starts as each 1MB DMA chunk
                    # lands instead of waiting for the full wq/x0
                    qps = []
                    for ct in range(4):
                        ps = apsp.tile([128, 512], F32, tag="qk", name="qps")
                        qps.append(ps)
                    for c in range(4):
                        for ct in range(4):
                            c0 = ct * 128
                            for kt in range(c * 8, c * 8 + 8):
                                nc.tensor.matmul(
                                    qps[ct][:], wq_sb[:, kt, c0:c0 + 128],
                                    x_sb[:, kt, :],
                                    start=(kt == 0), stop=(kt == KT - 1),
                                )
                    for ct in range(4):
                        c0 = ct * 128
                        st = astp.tile([128, 512], BF16, tag="qk_st")
                        nc.vector.tensor_scalar_add(
                            st[:], qps[ct][:], bq_sb[:, ct:ct + 1])
                        nc.sync.dma_start(
                            qT_d[0][c0:c0 + 128, 0:512], st[:])
                qk_cts = range(4, 8) if tck == 0 else range(8)
                for ct in qk_cts:
                    is_q = ct < 4
                    w_sb = wq_sb if is_q else wk_sb
                    b_sb = bq_sb if is_q else bk_sb
                    dst = qT_d[0] if is_q else kT_d[0]
                    c0 = (ct % 4) * 128
                    ps = apsp.tile([128, 512], F32, tag="qk")
                    for kt in range(KT):
                        nc.tensor.matmul(
                            ps[:], w_sb[:, kt, c0:c0 + 128], x_sb[:, kt, :],
                            start=(kt == 0), stop=(kt == KT - 1),
                        )
                    st = astp.tile([128, 512], BF16, tag="qk_st")
                    nc.vector.tensor_scalar_add(
                        st[:], ps[:], b_sb[:, ct % 4:ct % 4 + 1])
                    nc.sync.dma_start(dst[c0:c0 + 128, t0:t0 + 512], st[:])
                # v: [tok-tile 128, col 512], x stationary
                for tt in range(4):
                    ps = apsp.tile([128, 512], F32, tag="v")
                    for kt in range(KT):
                        nc.tensor.matmul(
                            ps[:], x_sb[:, kt, tt * 128:(tt + 1) * 128],
                            wv_sb[:, kt, :],
                            start=(kt == 0), stop=False,
                        )
                    nc.tensor.matmul(
                        ps[:], ones_bf[:, 0:128], bv_sb[:],
                        start=False, stop=True,
                    )
                    st = astp.tile([128, 512], BF16, tag="v_st")
                    nc.vector.tensor_copy(st[:], ps[:])
                    nc.sync.dma_start(
                        v_d[0][t0 + tt * 128:t0 + (tt + 1) * 128, :], st[:])
                if tck == 3:
                    # hoist (b0,h0) head loads: triggers enter the Sync queue
                    # here (data just became ready), not after all of seg1
                    load_head(bqk0, 0, 0, b0h0, "00")

        # ctxT per batch; b0's opens after seg0's pools release
        ctxp0 = ctx.enter_context(tc.tile_pool(name="ctxp0", bufs=1))
        ctxT0 = ctxp0.tile([128, HL, S], BF16)

        # ---------- shared denominator helpers ----------
        def emit_lbc(p, psump, bstp):
            lbc = psump.tile([128, 512], F32, tag="op", name="lbc_t")
            for j in range(2):
                nc.tensor.matmul(
                    lbc[:], onesmat[:], p["t2"][:, j, :],
                    start=(j == 0), stop=(j == 1))
            p["lbc"] = lbc

        def emit_norm(p, bstp):
            rec = bstp.tile([128, 512], F32, tag="rec", bufs=2, name="rec_t")
            nc.vector.reciprocal_approx_fast(out=rec[:], in_=p["lbc"][:])
            nc.vector.tensor_mul(p["dst"], p["cps"][:], rec[:])

        # ---------------- seg1: QKV for batch 1 + attention(b0) ----------------
        if "B" in phases:
         with tc.tile_pool(name="bqk1", bufs=1) as bqkp1, \
             tc.tile_pool(name="bpr1", bufs=1) as bprp1, \
             tc.tile_pool(name="bst1", bufs=1) as bstp1, \
             tc.tile_pool(name="aw1", bufs=1) as aw1, \
             tc.tile_pool(name="ax1", bufs=2) as ax1, \
             tc.tile_pool(name="ast1", bufs=3) as ast1, \
             tc.tile_pool(name="aps1", bufs=2, space="PSUM") as aps1, \
             tc.tile_pool(name="bsc1", bufs=3, space="PSUM") as bscp1, \
             tc.tile_pool(name="bcx1", bufs=2, space="PSUM") as bcxp1, \
             tc.tile_pool(name="lbp", bufs=1, space="PSUM") as lbp:

            # b0 attention piece stream: 128 pieces of (2 score MMs +
            # 2 512-wide exps + lagged PV pair + t4 accumulation)
            st1 = {"i": 0, "prev": None, "pending": None,
                   "head": None, "next_head": None, "t4": None, "cps": None}

            def b0_pv(pr):
                kj = pr["kj"]
                for u, pt in ((0, pr["pA"]), (1, pr["pB"])):
                    ki = 2 * kj + u
                    nc.tensor.matmul(
                        pr["cps"][:], pr["vh"][:, ki, :], pt[:],
                        start=(ki == 0), stop=(ki == NKT - 1),
                    )
                # progressive denominator fold into t4 (bf16)
                j = kj // 2
                if kj % 2 == 0:
                    nc.vector.tensor_add(
                        pr["t4"][:, j, :], pr["pA"][:], pr["pB"][:])
                else:
                    nc.vector.tensor_add(
                        pr["t4"][:, j, :], pr["t4"][:, j, :], pr["pA"][:])
                    nc.vector.tensor_add(
                        pr["t4"][:, j, :], pr["t4"][:, j, :], pr["pB"][:])
                if kj == 7:
                    t2 = bstp1.tile([128, 2, 512], BF16, tag="t2", bufs=2,
                                    name="t2s1")
                    nc.vector.tensor_add(
                        t2[:], pr["t4"][:, 0:4:2, :], pr["t4"][:, 1:4:2, :])
                    return {"t2": t2, "cps": pr["cps"], "dst": pr["dst"]}
                return None

            def b0_piece():
                i = st1["i"]
                if i >= 128:
                    return False
                hl, qc, pc = i // 32, (i // 8) % 4, i % 8
                if i == 0:
                    st1["head"] = dict(b0h0)
                if pc == 0:
                    if qc == 0 and hl > 0:
                        st1["head"] = st1["next_head"]
                    # new qc: fresh t4 / cps
                    t4 = bstp1.tile([128, 4, 512], BF16, tag="t4", bufs=2,
                                    name="t4s1")
                    st1["t4"] = t4
                    st1["cps"] = bcxp1.tile([128, 512], F32, tag="ctx",
                                            name="cps1")
                hd = st1["head"]
                q0 = qc * 512
                kj = pc
                pts = []
                for u in range(2):
                    ki = 2 * kj + u
                    sps = bscp1.tile([128, 512], F32, tag="sc", name="sps1")
                    nc.tensor.matmul(
                        sps[:], hd["kh"][:, ki * 128:(ki + 1) * 128],
                        hd["qh"][:, q0:q0 + 512], start=True, stop=True,
                    )
                    pt = bprp1.tile([128, 512], BF16, tag="probs", bufs=6,
                                    name="prb1")
                    nc.scalar.activation(pt[:], sps[:], AF.Exp, scale=SCALE)
                    pts.append(pt)
                if st1["prev"] is not None:
                    done = b0_pv(st1["prev"])
                    if done is not None:
                        st1["pending"] = done
                st1["prev"] = {
                    "kj": kj, "vh": hd["vh"], "pA": pts[0], "pB": pts[1],
                    "cps": st1["cps"], "t4": st1["t4"],
                    "dst": ctxT0[:, hl, q0:q0 + 512],
                }
                if pc == 2 and st1["pending"] is not None:
                    emit_lbc(st1["pending"], lbp, bstp1)
                elif pc == 5 and st1["pending"] is not None:
                    emit_norm(st1["pending"], bstp1)
                    st1["pending"] = None
                if hl < HL - 1 and qc == 3 and pc == 0:
                    nh = {}
                    load_head(bqkp1, 0, hl + 1, nh, f"0{hl + 1}")
                    st1["next_head"] = nh
                st1["i"] = i + 1
                return True

            def b0_drain():
                while b0_piece():
                    pass
                if st1["prev"] is not None:
                    done = b0_pv(st1["prev"])
                    if done is not None:
                        st1["pending"] = done
                    st1["prev"] = None
                if st1["pending"] is not None:
                    emit_lbc(st1["pending"], lbp, bstp1)
                    emit_norm(st1["pending"], bstp1)
                    st1["pending"] = None

            if "A" in phases:
                # weights reloaded (seg0's copies were released); the DMA time
                # hides under the first attention pieces
                wq2 = aw1.tile([128, KT, COLS], BF16)
                nc.sync.dma_start(
                    wq2[:], wq.rearrange("(kt p) c -> p kt c", p=128))
                wk2 = aw1.tile([128, KT, COLS], BF16)
                nc.sync.dma_start(
                    wk2[:], wk.rearrange("(kt p) c -> p kt c", p=128))
                wv2 = aw1.tile([128, KT, COLS], BF16)
                nc.sync.dma_start(
                    wv2[:], wv.rearrange("(kt p) c -> p kt c", p=128))

                groups_done = 0
                n_groups = 4 * 2 * (8 + 2)  # per tck: 2 halves x (8 qk + 2 v)

                def pace():
                    # stay ~16 pieces (2 q-chunks) ahead of the projection
                    # stream; the head start also covers the weight reload
                    target = min(128, (128 * groups_done) // n_groups + 16)
                    while st1["i"] < target:
                        if not b0_piece():
                            break

                pace()
                for tck in range(4, 8):
                    t0 = tck * 512
                    lt0 = t0 - S
                    # two half-token x tiles (N=256 groups), double buffered
                    for half in range(2):
                        h0 = t0 + half * 256
                        xh = ax1.tile([128, KT, 256], BF16, tag="x",
                                      name="xh1")
                        nc.sync.dma_start(
                            xh[:],
                            xT[:, h0:h0 + 256].rearrange(
                                "(kt p) t -> p kt t", p=128))
                        for ct in range(8):
                            is_q = ct < 4
                            w_sb = wq2 if is_q else wk2
                            b_sb = bq_sb if is_q else bk_sb
                            dst = qT_d[1] if is_q else kT_d[1]
                            c0 = (ct % 4) * 128
                            ps = aps1.tile([128, 256], F32, tag="qk",
                                           name="ps1")
                            for kt in range(KT):
                                nc.tensor.matmul(
                                    ps[:], w_sb[:, kt, c0:c0 + 128],
                                    xh[:, kt, :],
                                    start=(kt == 0), stop=(kt == KT - 1),
                                )
                            st = ast1.tile([128, 256], BF16, tag="qk_st",
                                           name="st1t")
                            nc.vector.tensor_scalar_add(
                                st[:], ps[:], b_sb[:, ct % 4:ct % 4 + 1])
                            nc.sync.dma_start(
                                dst[c0:c0 + 128,
                                    lt0 + half * 256:lt0 + half * 256 + 256],
                                st[:])
                            groups_done += 1
                            pace()
                        # v groups for this half (x stationary, N=512)
                        for tt in range(2):
                            ps = aps1.tile([128, 512], F32, tag="qk",
                                           name="ps1v")
                            for kt in range(KT):
                                nc.tensor.matmul(
                                    ps[:],
                                    xh[:, kt, tt * 128:(tt + 1) * 128],
                                    wv2[:, kt, :],
                                    start=(kt == 0), stop=False,
                                )
                            nc.tensor.matmul(
                                ps[:], ones_bf[:, 0:128], bv_sb[:],
                                start=False, stop=True,
                            )
                            st = ast1.tile([128, 512], BF16, tag="v_st",
                                           name="st1v")
                            nc.vector.tensor_copy(st[:], ps[:])
                            r0 = lt0 + half * 256 + tt * 128
                            nc.sync.dma_start(v_d[1][r0:r0 + 128, :], st[:])
                            groups_done += 1
                            pace()
                    if tck == 7:
                        # hoist (b1,h0) loads; bqk0's slots are free again
                        load_head(bqk0, 1, 0, b1h0, "10")
            b0_drain()

        # ---------------- seg2: attention(b1) + out-proj(b0) filler ----------------
        ctxp1 = ctx.enter_context(tc.tile_pool(name="ctxp1", bufs=1))
        ctxT1 = ctxp1.tile([128, HL, S], BF16)
        cwp = ctx.enter_context(tc.tile_pool(name="cw", bufs=1))
        wo_sb = cwp.tile([128, HL, H], BF16)
        nc.sync.dma_start(wo_sb[:], wo.rearrange("(hl p) n -> p hl n", p=128))

        def emit_ctile(ot, ncol, psump, sbufp, evict):
            t0, n0 = ot * 128, ncol * 512
            bb = t0 // S
            ctxb = ctxT0 if bb == 0 else ctxT1
            lt = t0 - bb * S
            ps = psump.tile([128, 512], F32, tag="op", name="cps_t")
            for hl in range(HL):
                nc.tensor.matmul(
                    ps[:], ctxb[:, hl, lt:lt + 128],
                    wo_sb[:, hl, n0:n0 + 512],
                    start=(hl == 0), stop=(hl == HL - 1),
                )
            st = sbufp.tile([128, 512], BF16, tag="ost", name="cst_t")
            if evict == 0:
                nc.scalar.copy(st[:], ps[:])
            else:
                nc.vector.tensor_copy(st[:], ps[:])
            nc.sync.dma_start(out[t0:t0 + 128, n0:n0 + 512], st[:])

        c_work = [(ot, ncol) for ot in range(S // 128)
                  for ncol in range(H // 512)]
        c_idx = 0

        if "B" in phases:
          with tc.tile_pool(name="bqk2", bufs=2) as bqkp2, \
             tc.tile_pool(name="bpr2", bufs=3) as bprp2, \
             tc.tile_pool(name="bst2", bufs=1) as bstp2, \
             tc.tile_pool(name="cst", bufs=4) as cstp, \
             tc.tile_pool(name="bsc2", bufs=2, space="PSUM") as bscp2, \
             tc.tile_pool(name="bcx2", bufs=2, space="PSUM") as bcxp2, \
             tc.tile_pool(name="cps", bufs=2, space="PSUM") as cpsp:

            pending = None
            prev = None

            def emit_pv(pr):
                kjp = pr["kj"]
                for u in range(2):
                    ki = 2 * kjp + u
                    nc.tensor.matmul(
                        pr["cps"][:], pr["vh"][:, ki, :],
                        pr["probs"][:, ki, :],
                        start=(ki == 0), stop=(ki == NKT - 1),
                    )
                nc.vector.tensor_add(
                    pr["tmp8"][:, kjp, :], pr["probs"][:, 2 * kjp, :],
                    pr["probs"][:, 2 * kjp + 1, :])
                if kjp == NKT // 2 - 1:
                    t4 = bstp2.tile([128, 4, 512], BF16, tag="t4", bufs=2)
                    nc.vector.tensor_add(
                        t4[:], pr["tmp8"][:, 0:8:2, :], pr["tmp8"][:, 1:8:2, :])
                    t2 = bstp2.tile([128, 2, 512], BF16, tag="t2", bufs=2)
                    nc.vector.tensor_add(
                        t2[:], t4[:, 0:4:2, :], t4[:, 1:4:2, :])
                    return {"t2": t2, "cps": pr["cps"], "dst": pr["dst"]}
                return None

            for hl in range(HL):
                if hl == 0 and b1h0:
                    qh, kh, vh = b1h0["qh"], b1h0["kh"], b1h0["vh"]
                else:
                    hd = {}
                    load_head(bqkp2, 1, hl, hd, f"1{hl}")
                    qh, kh, vh = hd["qh"], hd["kh"], hd["vh"]
                for qc in range(S // 512):
                    q0 = qc * 512
                    probs = bprp2.tile([128, NKT, 512], BF16, tag="probs")
                    tmp8 = bprp2.tile([128, NKT // 2, 512], BF16, tag="tmp8",
                                      bufs=2)
                    cps = bcxp2.tile([128, 512], F32, tag="ctx")
                    for kj in range(NKT // 2):
                        sps = bscp2.tile([128, 2, 512], F32, tag="sc")
                        for u in range(2):
                            ki = 2 * kj + u
                            nc.tensor.matmul(
                                sps[:, u, :],
                                kh[:, ki * 128:(ki + 1) * 128],
                                qh[:, q0:q0 + 512], start=True, stop=True,
                            )
                        nc.scalar.activation(
                            probs[:, 2 * kj:2 * kj + 2, :], sps[:],
                            AF.Exp, scale=SCALE)
                        if prev is not None:
                            done = emit_pv(prev)
                            if done is not None:
                                pending = done
                        prev = {
                            "kj": kj, "vh": vh, "probs": probs,
                            "tmp8": tmp8, "cps": cps,
                            "dst": ctxT1[:, hl, q0:q0 + 512],
                        }
                        if kj == 2 and pending is not None:
                            emit_lbc(pending, cpsp, bstp2)
                        elif kj == 5 and pending is not None:
                            emit_norm(pending, bstp2)
                            pending = None
                        elif kj in (3, 6) and c_idx < len(c_work):
                            ot, ncol = c_work[c_idx]
                            c_idx += 1
                            emit_ctile(ot, ncol, cpsp, cstp, 1)
            if prev is not None:
                done = emit_pv(prev)
                if done is not None:
                    pending = done
                prev = None
            if pending is not None:
                emit_lbc(pending, cpsp, bstp2)
                emit_norm(pending, bstp2)
                pending = None

        # ---------------- seg3: out-projection tail ----------------
        if "C" in phases:
          with tc.tile_pool(name="cst2", bufs=8) as cst2, \
               tc.tile_pool(name="cps2", bufs=6, space="PSUM") as cpsp2:
            tail = c_work[c_idx:] + [
                (ot, ncol) for ot in range(S // 128, TOK // 128)
                for ncol in range(H // 512)]
            for i, (ot, ncol) in enumerate(tail):
                emit_ctile(ot, ncol, cpsp2, cst2, i % 2)

    nc.compile()
    return nc


_NC = None


def _get_nc():
    global _NC
    if _NC is None:
        _NC = build_nc()
    return _NC


def _shard_inputs(x, attn_qkvw, attn_qkvb, attn_ow):
    bf = ml_dtypes.bfloat16
    x = np.asarray(x, dtype=np.float32)
    w = np.asarray(attn_qkvw, dtype=np.float32)
    b = np.asarray(attn_qkvb, dtype=np.float32)
    wo = np.asarray(attn_ow, dtype=np.float32)

    xT = np.ascontiguousarray(x.reshape(TOK, H).T).astype(bf)
    w4 = w.reshape(H, 3, 32, HD)
    b4 = b.reshape(3, 32, HD)
    in_maps = []
    for c in range(NCORES):
        hs = slice(c * HL, (c + 1) * HL)
        in_maps.append({
            "xT": xT,
            "wq": np.ascontiguousarray(w4[:, 0, hs, :].reshape(H, COLS)).astype(bf),
            "wk": np.ascontiguousarray(w4[:, 1, hs, :].reshape(H, COLS)).astype(bf),
            "wv": np.ascontiguousarray(w4[:, 2, hs, :].reshape(H, COLS)).astype(bf),
            "bq": np.ascontiguousarray(b4[0, hs, :].reshape(1, COLS)),
            "bk": np.ascontiguousarray(b4[1, hs, :].reshape(1, COLS)),
            "bv": b4[2, hs, :].reshape(1, COLS).astype(bf),
            "wo": np.ascontiguousarray(
                wo[c * COLS:(c + 1) * COLS, :]).astype(bf),
        })
    return in_maps


def kernel(x, attn_qkvw, attn_qkvb, attn_ow, attn_ob):
    import time as _time
    nc = _get_nc()
    in_maps = _shard_inputs(x, attn_qkvw, attn_qkvb, attn_ow)
    res = None
    for attempt in range(3):
        try:
            res = run_bass_kernel_spmd(nc, in_maps, core_ids=list(range(NCORES)))
            break
        except Exception:
            # transient NRT_EXEC_UNIT_UNRECOVERABLE has been observed on a
            # first dispatch; rebuild and retry once before giving up
            if attempt == 2:
                raise
            _time.sleep(2)
            global _NC
            _NC = None
            nc = _get_nc()
    acc = np.asarray(res.results[0]["out"], dtype=np.float32)
    for c in range(1, NCORES):
        acc = acc + np.asarray(res.results[c]["out"], dtype=np.float32)
    acc = acc + np.asarray(attn_ob, dtype=np.float32)[None, :]
    return acc.reshape(NB, S, H)


# revision 24
# speedup vs baseline: 1.0204x; 1.0204x over previous
"""DeepSpeedAttention (B=2, S=2048, H=4096, 32 heads) on 8 Trainium2 cores.

Sharding: tensor-parallel across heads. Each core computes QKV for its 4
heads (column shard of attn_qkvw), full attention for those heads, and a
partial output projection (row shard of attn_ow). The 8 bf16 partial outputs
are summed on the host (host-side all-reduce) in f32 and the bias is added.

Schedule (per core) - four segments, engineered so TensorE never waits on
the ACT-bound softmax:
  seg0: QKV projection for batch 0 (token chunks 0-3). Classic weight-
        stationary sweep; warmup matmuls + chunked first DMAs hide HAM
        cold-start; (b0,h0) attention head loads are emitted here.
  seg1: QKV projection for batch 1 FUSED with the whole of batch-0
        attention: the attention work is emitted as small "pieces"
        (2 k-tiles: scores matmuls + 512-wide exp + lagged PV) between
        projection matmul groups, so exp/DVE latency hides under dense
        independent projection work. x is streamed in half-token (N=256)
        double-buffered tiles to make room for resident wq/wk/wv.
  seg2: batch-1 attention (classic 1024-wide exp, PV lagged one k-pair
        globally) with batch-0 out-projection tiles interleaved as PE
        filler.
  seg3: remaining out-projection tail on a deep PSUM ring (matmul-bound).
Softmax denominator: probs pair-sums fold progressively into t4/t2 on DVE
(bf16), then one ones-matrix matmul group does the 128-partition
reduce+broadcast; reciprocal+scale pipelined into the next chunk.
"""

import os
import numpy as np
import ml_dtypes
from contextlib import ExitStack

try:
    import jax
    jax.config.update(
        "jax_compilation_cache_dir", os.path.expanduser("~/.bass_jax_cache"))
    jax.config.update("jax_persistent_cache_min_compile_time_secs", 10.0)
    jax.config.update("jax_persistent_cache_min_entry_size_bytes", 0)
except Exception:
    pass

import concourse.bass as bass
from concourse import bass_isa
import concourse.tile as tile
from concourse import bacc, mybir
from concourse.bass_utils import run_bass_kernel_spmd

BF16 = mybir.dt.bfloat16
F32 = mybir.dt.float32
AF = mybir.ActivationFunctionType

H = 4096          # hidden
TOK = 4096        # B*S tokens
S = 2048          # seq len per batch
NB = 2            # batches
HL = 4            # heads per core
HD = 128          # head dim
COLS = HL * HD    # per-core hidden shard (512)
NCORES = 8
KT = H // 128     # 32 contraction tiles for the projections
NKT = S // 128    # 16 k tiles per batch
SCALE = 1.0 / float(np.sqrt(HD))


def build_nc(phases: str = "ABC"):
    nc = bacc.Bacc("TRN2", target_bir_lowering=False, debug=False)

    xT = nc.dram_tensor("xT", [H, TOK], BF16, kind="ExternalInput").ap()
    wq = nc.dram_tensor("wq", [H, COLS], BF16, kind="ExternalInput").ap()
    wk = nc.dram_tensor("wk", [H, COLS], BF16, kind="ExternalInput").ap()
    wv = nc.dram_tensor("wv", [H, COLS], BF16, kind="ExternalInput").ap()
    bq = nc.dram_tensor("bq", [1, COLS], F32, kind="ExternalInput").ap()
    bk = nc.dram_tensor("bk", [1, COLS], F32, kind="ExternalInput").ap()
    bv = nc.dram_tensor("bv", [1, COLS], BF16, kind="ExternalInput").ap()
    wo = nc.dram_tensor("wo", [COLS, H], BF16, kind="ExternalInput").ap()
    # bf16 partials (summed in f32 on host): halves out-DMA + eviction cost
    out = nc.dram_tensor("out", [TOK, H], BF16, kind="ExternalOutput").ap()

    with tile.TileContext(nc) as tc, ExitStack() as ctx:
        # staging split per batch so batch-0 head loads can start mid-phase-A
        dram = ctx.enter_context(tc.tile_pool(name="dram", bufs=1, space="DRAM"))
        qT_d = [dram.tile([COLS, S], BF16, name=f"qT_d{b}") for b in range(NB)]
        kT_d = [dram.tile([COLS, S], BF16, name=f"kT_d{b}") for b in range(NB)]
        v_d = [dram.tile([S, COLS], BF16, name=f"v_d{b}") for b in range(NB)]

        const = ctx.enter_context(tc.tile_pool(name="const", bufs=1))
        ones_bf = const.tile([1, 512], BF16)
        nc.vector.memset(ones_bf[:], 1.0)
        # [128,128] all-ones: one matmul does the 128-partition denominator
        # reduce AND broadcasts the sum to every output partition
        onesmat = const.tile([128, 128], BF16)
        nc.vector.memset(onesmat[:], 1.0)
        warm_sb = const.tile([128, 128], BF16)
        nc.vector.memset(warm_sb[:], 0.125)
        # per-partition layout [col-within-tile, col-tile] for tensor_scalar
        bq_sb = const.tile([128, HL], F32)
        nc.sync.dma_start(bq_sb[:], bq.rearrange("o (ct p) -> p (o ct)", p=128))
        bk_sb = const.tile([128, HL], F32)
        nc.sync.dma_start(bk_sb[:], bk.rearrange("o (ct p) -> p (o ct)", p=128))
        bv_sb = const.tile([1, COLS], BF16)
        nc.sync.dma_start(bv_sb[:], bv)

        # head-tile pool that survives segment boundaries: (b0,h0) loads are
        # emitted mid-seg0, (b1,h0) loads mid-seg1
        bqk0 = ctx.enter_context(tc.tile_pool(name="bqk0", bufs=1))
        b0h0 = {}
        b1h0 = {}

        def load_head(pool, b, hl, store, suffix):
            r0 = hl * 128
            qh = pool.tile([128, S], BF16, tag="qh", name=f"qh{suffix}")
            nc.sync.dma_start(qh[:], qT_d[b][r0:r0 + 128, :])
            kh = pool.tile([128, S], BF16, tag="kh", name=f"kh{suffix}")
            nc.sync.dma_start(kh[:], kT_d[b][r0:r0 + 128, :])
            vh = pool.tile([128, NKT, 128], BF16, tag="vh", name=f"vh{suffix}")
            nc.sync.dma_start(
                vh[:],
                v_d[b][:, r0:r0 + 128].rearrange("(i p) d -> p i d", p=128))
            store.clear()
            store.update({"qh": qh, "kh": kh, "vh": vh})

        # ---------------- seg0: QKV for batch 0 ----------------
        if "A" in phases:
         with tc.tile_pool(name="aw", bufs=1) as awp, \
             tc.tile_pool(name="ax", bufs=2) as axp, \
             tc.tile_pool(name="ast", bufs=6) as astp, \
             tc.tile_pool(name="aps", bufs=4, space="PSUM") as apsp:
            # PE warmup: dead back-to-back matmuls during the initial DMA
            # wait get HAM to K=8/8 before the first real matmul issues
            wps = apsp.tile([128, 512], F32, tag="qk")
            for _ in range(64):
                nc.tensor.matmul(
                    wps[:, 0:128], warm_sb[:], warm_sb[:], start=True, stop=True)
            # wq + first x chunk first, in kt-chunks so the first q matmuls
            # can start after ~1/4 of the bytes land (subtile deps)
            wq_sb = awp.tile([128, KT, COLS], BF16)
            x0_sb = axp.tile([128, KT, 512], BF16, tag="x")
            for c in range(4):
                r0, r1, k0, k1 = c * 1024, (c + 1) * 1024, c * 8, (c + 1) * 8
                nc.sync.dma_start(
                    wq_sb[:, k0:k1, :],
                    wq[r0:r1, :].rearrange("(kt p) c -> p kt c", p=128))
                nc.sync.dma_start(
                    x0_sb[:, k0:k1, :],
                    xT[r0:r1, 0:512].rearrange("(kt p) t -> p kt t", p=128))
            wk_sb = awp.tile([128, KT, COLS], BF16)
            nc.sync.dma_start(wk_sb[:], wk.rearrange("(kt p) c -> p kt c", p=128))
            wv_sb = awp.tile([128, KT, COLS], BF16)
            nc.sync.dma_start(wv_sb[:], wv.rearrange("(kt p) c -> p kt c", p=128))

            for tck in range(4):
                t0 = tck * 512
                if tck == 0:
                    x_sb = x0_sb
                else:
                    x_sb = axp.tile([128, KT, 512], BF16, tag="x")
                    nc.sync.dma_start(
                        x_sb[:],
                        xT[:, t0:t0 + 512].rearrange("(kt p) t -> p kt t", p=128),
                    )
                # qT / kT: [col-tile 128, tok 512], weight stationary
                if tck == 0:
                    # kt-chunk-major for the q groups: 4 accumulation groups
                    # open at once so compute starts as each 1MB DMA chunk
                    # lands instead of waiting for the full wq/x0
                    qps = []
                    for ct in range(4):
                        ps = apsp.tile([128, 512], F32, tag="qk", name="qps")
                        qps.append(ps)
                    for c in range(4):
                        for ct in range(4):
                            c0 = ct * 128
                            for kt in range(c * 8, c * 8 + 8):
                                nc.tensor.matmul(
                                    qps[ct][:], wq_sb[:, kt, c0:c0 + 128],
                                    x_sb[:, kt, :],
                                    start=(kt == 0), stop=(kt == KT - 1),
                                )
                    for ct in range(4):
                        c0 = ct * 128
                        st = astp.tile([128, 512], BF16, tag="qk_st")
                        nc.vector.tensor_scalar_add(
                            st[:], qps[ct][:], bq_sb[:, ct:ct + 1])
                        nc.sync.dma_start(
                            qT_d[0][c0:c0 + 128, 0:512], st[:])
                qk_cts = range(4, 8) if tck == 0 else range(8)
                for ct in qk_cts:
                    is_q = ct < 4
                    w_sb = wq_sb if is_q else wk_sb
                    b_sb = bq_sb if is_q else bk_sb
                    dst = qT_d[0] if is_q else kT_d[0]
                    c0 = (ct % 4) * 128
                    ps = apsp.tile([128, 512], F32, tag="qk")
                    for kt in range(KT):
                        nc.tensor.matmul(
                            ps[:], w_sb[:, kt, c0:c0 + 128], x_sb[:, kt, :],
                            start=(kt == 0), stop=(kt == KT - 1),
                        )
                    st = astp.tile([128, 512], BF16, tag="qk_st")
                    nc.vector.tensor_scalar_add(
                        st[:], ps[:], b_sb[:, ct % 4:ct % 4 + 1])
                    nc.sync.dma_start(dst[c0:c0 + 128, t0:t0 + 512], st[:])
                # v: [tok-tile 128, col 512], x stationary
                for tt in range(4):
                    ps = apsp.tile([128, 512], F32, tag="v")
                    for kt in range(KT):
                        nc.tensor.matmul(
                            ps[:], x_sb[:, kt, tt * 128:(tt + 1) * 128],
                            wv_sb[:, kt, :],
                            start=(kt == 0), stop=False,
                        )
                    nc.tensor.matmul(
                        ps[:], ones_bf[:, 0:128], bv_sb[:],
                        start=False, stop=True,
                    )
                    st = astp.tile([128, 512], BF16, tag="v_st")
                    nc.vector.tensor_copy(st[:], ps[:])
                    nc.sync.dma_start(
                        v_d[0][t0 + tt * 128:t0 + (tt + 1) * 128, :], st[:])
                if tck == 3:
                    # hoist (b0,h0) head loads: triggers enter the Sync queue
                    # here (data just became ready), not after all of seg1
                    load_head(bqk0, 0, 0, b0h0, "00")

        # ctxT per batch; b0's opens after seg0's pools release
        ctxp0 = ctx.enter_context(tc.tile_pool(name="ctxp0", bufs=1))
        ctxT0 = ctxp0.tile([128, HL, S], BF16)

        # ---------- shared denominator helpers ----------
        def emit_lbc(p, psump, bstp):
            lbc = psump.tile([128, 512], F32, tag="op", name="lbc_t")
            for j in range(2):
                nc.tensor.matmul(
                    lbc[:], onesmat[:], p["t2"][:, j, :],
                    start=(j == 0), stop=(j == 1))
            p["lbc"] = lbc

        def emit_norm(p, bstp):
            rec = bstp.tile([128, 512], F32, tag="rec", bufs=2, name="rec_t")
            nc.vector.reciprocal_approx_fast(out=rec[:], in_=p["lbc"][:])
            nc.vector.tensor_mul(p["dst"], p["cps"][:], rec[:])

        # ---------------- seg1: QKV for batch 1 + attention(b0) ----------------
        if "B" in phases:
         with tc.tile_pool(name="bqk1", bufs=1) as bqkp1, \
             tc.tile_pool(name="bpr1", bufs=1) as bprp1, \
             tc.tile_pool(name="bst1", bufs=1) as bstp1, \
             tc.tile_pool(name="aw1", bufs=1) as aw1, \
             tc.tile_pool(name="ax1", bufs=2) as ax1, \
             tc.tile_pool(name="ast1", bufs=3) as ast1, \
             tc.tile_pool(name="aps1", bufs=2, space="PSUM") as aps1, \
             tc.tile_pool(name="bsc1", bufs=3, space="PSUM") as bscp1, \
             tc.tile_pool(name="bcx1", bufs=2, space="PSUM") as bcxp1, \
             tc.tile_pool(name="lbp", bufs=1, space="PSUM") as lbp:

            # b0 attention piece stream: 128 pieces of (2 score MMs +
            # 2 512-wide exps + lagged PV pair + t4 accumulation)
            st1 = {"i": 0, "prev": None, "pending": None,
                   "head": None, "next_head": None, "t4": None, "cps": None}

            def b0_pv(pr):
                kj = pr["kj"]
                for u, pt in ((0, pr["pA"]), (1, pr["pB"])):
                    ki = 2 * kj + u
                    nc.tensor.matmul(
                        pr["cps"][:], pr["vh"][:, ki, :], pt[:],
                        start=(ki == 0), stop=(ki == NKT - 1),
                    )
                # progressive denominator fold into t4 (bf16)
                j = kj // 2
                if kj % 2 == 0:
                    nc.vector.tensor_add(
                        pr["t4"][:, j, :], pr["pA"][:], pr["pB"][:])
                else:
                    nc.vector.tensor_add(
                        pr["t4"][:, j, :], pr["t4"][:, j, :], pr["pA"][:])
                    nc.vector.tensor_add(
                        pr["t4"][:, j, :], pr["t4"][:, j, :], pr["pB"][:])
                if kj == 7:
                    t2 = bstp1.tile([128, 2, 512], BF16, tag="t2", bufs=2,
                                    name="t2s1")
                    nc.vector.tensor_add(
                        t2[:], pr["t4"][:, 0:4:2, :], pr["t4"][:, 1:4:2, :])
                    return {"t2": t2, "cps": pr["cps"], "dst": pr["dst"]}
                return None

            def b0_piece():
                i = st1["i"]
                if i >= 128:
                    return False
                hl, qc, pc = i // 32, (i // 8) % 4, i % 8
                if i == 0:
                    st1["head"] = dict(b0h0)
                if pc == 0:
                    if qc == 0 and hl > 0:
                        st1["head"] = st1["next_head"]
                    # new qc: fresh t4 / cps
                    t4 = bstp1.tile([128, 4, 512], BF16, tag="t4", bufs=2,
                                    name="t4s1")
                    st1["t4"] = t4
                    st1["cps"] = bcxp1.tile([128, 512], F32, tag="ctx",
                                            name="cps1")
                hd = st1["head"]
                q0 = qc * 512
                kj = pc
                pts = []
                for u in range(2):
                    ki = 2 * kj + u
                    sps = bscp1.tile([128, 512], F32, tag="sc", name="sps1")
                    nc.tensor.matmul(
                        sps[:], hd["kh"][:, ki * 128:(ki + 1) * 128],
                        hd["qh"][:, q0:q0 + 512], start=True, stop=True,
                    )
                    pt = bprp1.tile([128, 512], BF16, tag="probs", bufs=6,
                                    name="prb1")
                    nc.scalar.activation(pt[:], sps[:], AF.Exp, scale=SCALE)
                    pts.append(pt)
                if st1["prev"] is not None:
                    done = b0_pv(st1["prev"])
                    if done is not None:
                        st1["pending"] = done
                st1["prev"] = {
                    "kj": kj, "vh": hd["vh"], "pA": pts[0], "pB": pts[1],
                    "cps": st1["cps"], "t4": st1["t4"],
                    "dst": ctxT0[:, hl, q0:q0 + 512],
                }
                if pc == 2 and st1["pending"] is not None:
                    emit_lbc(st1["pending"], lbp, bstp1)
                elif pc == 5 and st1["pending"] is not None:
                    emit_norm(st1["pending"], bstp1)
                    st1["pending"] = None
                if hl < HL - 1 and qc == 3 and pc == 0:
                    nh = {}
                    load_head(bqkp1, 0, hl + 1, nh, f"0{hl + 1}")
                    st1["next_head"] = nh
                st1["i"] = i + 1
                return True

            def b0_drain():
                while b0_piece():
                    pass
                if st1["prev"] is not None:
                    done = b0_pv(st1["prev"])
                    if done is not None:
                        st1["pending"] = done
                    st1["prev"] = None
                if st1["pending"] is not None:
                    emit_lbc(st1["pending"], lbp, bstp1)
                    emit_norm(st1["pending"], bstp1)
                    st1["pending"] = None

            if "A" in phases:
                # weights reloaded (seg0's copies were released); the DMA time
                # hides under the first attention pieces
                wq2 = aw1.tile([128, KT, COLS], BF16)
                for c in range(4):
                    r0, r1, k0, k1 = c * 1024, (c + 1) * 1024, c * 8, (c + 1) * 8
                    nc.sync.dma_start(
                        wq2[:, k0:k1, :],
                        wq[r0:r1, :].rearrange("(kt p) c -> p kt c", p=128))
                wk2 = aw1.tile([128, KT, COLS], BF16)
                nc.sync.dma_start(
                    wk2[:], wk.rearrange("(kt p) c -> p kt c", p=128))
                wv2 = aw1.tile([128, KT, COLS], BF16)
                nc.sync.dma_start(
                    wv2[:], wv.rearrange("(kt p) c -> p kt c", p=128))

                groups_done = 0
                n_groups = 4 * 2 * (8 + 2)  # per tck: 2 halves x (8 qk + 2 v)

                def pace():
                    # head start covers the weight reload; the rest spreads
                    # so pieces and projection groups finish together
                    target = min(128, (96 * groups_done) // n_groups + 32)
                    while st1["i"] < target:
                        if not b0_piece():
                            break

                pace()
                for tck in range(4, 8):
                    t0 = tck * 512
                    lt0 = t0 - S
                    # two half-token x tiles (N=256 groups), double buffered
                    for half in range(2):
                        h0 = t0 + half * 256
                        xh = ax1.tile([128, KT, 256], BF16, tag="x",
                                      name="xh1")
                        nc.sync.dma_start(
                            xh[:],
                            xT[:, h0:h0 + 256].rearrange(
                                "(kt p) t -> p kt t", p=128))
                        for ct in range(8):
                            is_q = ct < 4
                            w_sb = wq2 if is_q else wk2
                            b_sb = bq_sb if is_q else bk_sb
                            dst = qT_d[1] if is_q else kT_d[1]
                            c0 = (ct % 4) * 128
                            ps = aps1.tile([128, 256], F32, tag="qk",
                                           name="ps1")
                            for kt in range(KT):
                                nc.tensor.matmul(
                                    ps[:], w_sb[:, kt, c0:c0 + 128],
                                    xh[:, kt, :],
                                    start=(kt == 0), stop=(kt == KT - 1),
                                )
                            st = ast1.tile([128, 256], BF16, tag="qk_st",
                                           name="st1t")
                            nc.vector.tensor_scalar_add(
                                st[:], ps[:], b_sb[:, ct % 4:ct % 4 + 1])
                            nc.sync.dma_start(
                                dst[c0:c0 + 128,
                                    lt0 + half * 256:lt0 + half * 256 + 256],
                                st[:])
                            groups_done += 1
                            pace()
                        # v groups for this half (x stationary, N=512)
                        for tt in range(2):
                            ps = aps1.tile([128, 512], F32, tag="qk",
                                           name="ps1v")
                            for kt in range(KT):
                                nc.tensor.matmul(
                                    ps[:],
                                    xh[:, kt, tt * 128:(tt + 1) * 128],
                                    wv2[:, kt, :],
                                    start=(kt == 0), stop=False,
                                )
                            nc.tensor.matmul(
                                ps[:], ones_bf[:, 0:128], bv_sb[:],
                                start=False, stop=True,
                            )
                            st = ast1.tile([128, 512], BF16, tag="v_st",
                                           name="st1v")
                            nc.vector.tensor_copy(st[:], ps[:])
                            r0 = lt0 + half * 256 + tt * 128
                            nc.sync.dma_start(v_d[1][r0:r0 + 128, :], st[:])
                            groups_done += 1
                            pace()
                    if tck == 7:
                        # hoist (b1,h0) loads; bqk0's slots are free again
                        load_head(bqk0, 1, 0, b1h0, "10")
            b0_drain()

        # ---------------- seg2: attention(b1) + out-proj(b0) filler ----------------
        ctxp1 = ctx.enter_context(tc.tile_pool(name="ctxp1", bufs=1))
        ctxT1 = ctxp1.tile([128, HL, S], BF16)
        cwp = ctx.enter_context(tc.tile_pool(name="cw", bufs=1))
        wo_sb = cwp.tile([128, HL, H], BF16)
        nc.sync.dma_start(wo_sb[:], wo.rearrange("(hl p) n -> p hl n", p=128))

        def emit_ctile(ot, ncol, psump, sbufp, evict):
            t0, n0 = ot * 128, ncol * 512
            bb = t0 // S
            ctxb = ctxT0 if bb == 0 else ctxT1
            lt = t0 - bb * S
            ps = psump.tile([128, 512], F32, tag="op", name="cps_t")
            for hl in range(HL):
                nc.tensor.matmul(
                    ps[:], ctxb[:, hl, lt:lt + 128],
                    wo_sb[:, hl, n0:n0 + 512],
                    start=(hl == 0), stop=(hl == HL - 1),
                )
            st = sbufp.tile([128, 512], BF16, tag="ost", name="cst_t")
            if evict == 0:
                nc.scalar.copy(st[:], ps[:])
            else:
                nc.vector.tensor_copy(st[:], ps[:])
            nc.sync.dma_start(out[t0:t0 + 128, n0:n0 + 512], st[:])

        c_work = [(ot, ncol) for ot in range(S // 128)
                  for ncol in range(H // 512)]
        c_idx = 0

        if "B" in phases:
          with tc.tile_pool(name="bqk2", bufs=2) as bqkp2, \
             tc.tile_pool(name="bpr2", bufs=3) as bprp2, \
             tc.tile_pool(name="bst2", bufs=1) as bstp2, \
             tc.tile_pool(name="cst", bufs=4) as cstp, \
             tc.tile_pool(name="bsc2", bufs=2, space="PSUM") as bscp2, \
             tc.tile_pool(name="bcx2", bufs=2, space="PSUM") as bcxp2, \
             tc.tile_pool(name="cps", bufs=2, space="PSUM") as cpsp:

            pending = None
            prev = None

            def emit_pv(pr):
                kjp = pr["kj"]
                for u in range(2):
                    ki = 2 * kjp + u
                    nc.tensor.matmul(
                        pr["cps"][:], pr["vh"][:, ki, :],
                        pr["probs"][:, ki, :],
                        start=(ki == 0), stop=(ki == NKT - 1),
                    )
                nc.vector.tensor_add(
                    pr["tmp8"][:, kjp, :], pr["probs"][:, 2 * kjp, :],
                    pr["probs"][:, 2 * kjp + 1, :])
                if kjp == NKT // 2 - 1:
                    t4 = bstp2.tile([128, 4, 512], BF16, tag="t4", bufs=2)
                    nc.vector.tensor_add(
                        t4[:], pr["tmp8"][:, 0:8:2, :], pr["tmp8"][:, 1:8:2, :])
                    t2 = bstp2.tile([128, 2, 512], BF16, tag="t2", bufs=2)
                    nc.vector.tensor_add(
                        t2[:], t4[:, 0:4:2, :], t4[:, 1:4:2, :])
                    return {"t2": t2, "cps": pr["cps"], "dst": pr["dst"]}
                return None

            for hl in range(HL):
                if hl == 0 and b1h0:
                    qh, kh, vh = b1h0["qh"], b1h0["kh"], b1h0["vh"]
                else:
                    hd = {}
                    load_head(bqkp2, 1, hl, hd, f"1{hl}")
                    qh, kh, vh = hd["qh"], hd["kh"], hd["vh"]
                for qc in range(S // 512):
                    q0 = qc * 512
                    probs = bprp2.tile([128, NKT, 512], BF16, tag="probs")
                    tmp8 = bprp2.tile([128, NKT // 2, 512], BF16, tag="tmp8",
                                      bufs=2)
                    cps = bcxp2.tile([128, 512], F32, tag="ctx")
                    for kj in range(NKT // 2):
                        sps = bscp2.tile([128, 2, 512], F32, tag="sc")
                        for u in range(2):
                            ki = 2 * kj + u
                            nc.tensor.matmul(
                                sps[:, u, :],
                                kh[:, ki * 128:(ki + 1) * 128],
                                qh[:, q0:q0 + 512], start=True, stop=True,
                            )
                        nc.scalar.activation(
                            probs[:, 2 * kj:2 * kj + 2, :], sps[:],
                            AF.Exp, scale=SCALE)
                        if prev is not None:
                            done = emit_pv(prev)
                            if done is not None:
                                pending = done
                        prev = {
                            "kj": kj, "vh": vh, "probs": probs,
                            "tmp8": tmp8, "cps": cps,
                            "dst": ctxT1[:, hl, q0:q0 + 512],
                        }
                        if kj == 2 and pending is not None:
                            emit_lbc(pending, cpsp, bstp2)
                        elif kj == 5 and pending is not None:
                            emit_norm(pending, bstp2)
                            pending = None
                        elif kj in (3, 6) and c_idx < len(c_work):
                            ot, ncol = c_work[c_idx]
                            c_idx += 1
                            emit_ctile(ot, ncol, cpsp, cstp, 1)
            if prev is not None:
                done = emit_pv(prev)
                if done is not None:
                    pending = done
                prev = None
            if pending is not None:
                emit_lbc(pending, cpsp, bstp2)
                emit_norm(pending, bstp2)
                pending = None

        # ---------------- seg3: out-projection tail ----------------
        if "C" in phases:
          with tc.tile_pool(name="cst2", bufs=8) as cst2, \
               tc.tile_pool(name="cps2", bufs=6, space="PSUM") as cpsp2:
            tail = c_work[c_idx:] + [
                (ot, ncol) for ot in range(S // 128, TOK // 128)
                for ncol in range(H // 512)]
            for i, (ot, ncol) in enumerate(tail):
                emit_ctile(ot, ncol, cpsp2, cst2, i % 2)

    nc.compile()
    return nc


_NC = None


def _get_nc():
    global _NC
    if _NC is None:
        _NC = build_nc()
    return _NC


def _shard_inputs(x, attn_qkvw, attn_qkvb, attn_ow):
    bf = ml_dtypes.bfloat16
    x = np.asarray(x, dtype=np.float32)
    w = np.asarray(attn_qkvw, dtype=np.float32)
    b = np.asarray(attn_qkvb, dtype=np.float32)
    wo = np.asarray(attn_ow, dtype=np.float32)

    xT = np.ascontiguousarray(x.reshape(TOK, H).T).astype(bf)
    w4 = w.reshape(H, 3, 32, HD)
    b4 = b.reshape(3, 32, HD)
    in_maps = []
    for c in range(NCORES):
        hs = slice(c * HL, (c + 1) * HL)
        in_maps.append({
            "xT": xT,
            "wq": np.ascontiguousarray(w4[:, 0, hs, :].reshape(H, COLS)).astype(bf),
            "wk": np.ascontiguousarray(w4[:, 1, hs, :].reshape(H, COLS)).astype(bf),
            "wv": np.ascontiguousarray(w4[:, 2, hs, :].reshape(H, COLS)).astype(bf),
            "bq": np.ascontiguousarray(b4[0, hs, :].reshape(1, COLS)),
            "bk": np.ascontiguousarray(b4[1, hs, :].reshape(1, COLS)),
            "bv": b4[2, hs, :].reshape(1, COLS).astype(bf),
            "wo": np.ascontiguousarray(
                wo[c * COLS:(c + 1) * COLS, :]).astype(bf),
        })
    return in_maps


def kernel(x, attn_qkvw, attn_qkvb, attn_ow, attn_ob):
    import time as _time
    nc = _get_nc()
    in_maps = _shard_inputs(x, attn_qkvw, attn_qkvb, attn_ow)
    res = None
    for attempt in range(3):
        try:
            res = run_bass_kernel_spmd(nc, in_maps, core_ids=list(range(NCORES)))
            break
        except Exception:
            # transient NRT_EXEC_UNIT_UNRECOVERABLE has been observed on a
            # first dispatch; rebuild and retry once before giving up
            if attempt == 2:
                raise
            _time.sleep(2)
            global _NC
            _NC = None
            nc = _get_nc()
    acc = np.asarray(res.results[0]["out"], dtype=np.float32)
    for c in range(1, NCORES):
        acc = acc + np.asarray(res.results[c]["out"], dtype=np.float32)
    acc = acc + np.asarray(attn_ob, dtype=np.float32)[None, :]
    return acc.reshape(NB, S, H)
